# revision 8
# baseline (speedup 1.0000x reference)
"""Trainium2 Bass kernel for nn_PointCloud2LaserScanLoss.

Problem (per batch element b of B=8):
    d2[n,m] = ||pred[n] - targ[m]||^2          (N=M=4096, D=2)
    minval[n] = min over valid m (m < tp_b) of d2[n,m]
    coord_b   = sum over valid n (n < pp_b) of minval[n] / (pp_b * D)
    coord     = mean_b coord_b
    points    = mean_b ((pp_b - tp_b)/N)^2
    total     = coord + 0.1 * points
(The reference gathers matched targets and recomputes the squared distance,
which is numerically the min of the masked distance row — so no argmin/gather
is needed, just a masked min-reduction.)

Sharding: data-parallel over batch; core b handles batch element b.

Device algorithm per core:
  - d2 is computed on the TensorEngine as an 18-row augmented matmul in bf16
    using an exact 3-way bf16 split of every coordinate (hi/mid/lo), which
    reproduces fp32-grade precision at full bf16 PE speed (fp32 matmul is 4x
    slower).  The target-validity mask is baked into the |t|^2 rows (1e30 for
    invalid targets).
  - The [128, 4096] distance rows per n-tile land in PSUM as four [128,1024]
    quarters.  ScalarE copies two quarters to SBUF; VectorE then runs
    tensor_tensor_reduce (elementwise min of a PSUM quarter and an SBUF
    quarter, fused with a min-reduction) so the DVE drains 2 elements per
    lane-cycle instead of 1.
  - Epilogue on device: pair-min, predicted-validity mask multiply,
    free-axis sum, then a [128,1]x[128,1] matmul to sum over partitions.
Host combines the 8 per-core masked sums into the three scalar losses.
"""

import sys

import numpy as np

if "/opt/trn_rl_repo" not in sys.path:
    sys.path.insert(0, "/opt/trn_rl_repo")

import ml_dtypes

B, N, M, D = 8, 4096, 4096, 2
NT = N // 128  # 32 n-tiles
K = 18  # augmented contraction rows
BIG = 1e30

_BF16 = ml_dtypes.bfloat16

_compiled = None  # cached (nc, core_ids)


def _split3(v64):
    """Exact-ish 3-way bf16 split: v ~= h + m + l with residual ~2^-27 |v|."""
    h = v64.astype(_BF16)
    r = v64 - h.astype(np.float64)
    m = r.astype(_BF16)
    r2 = r - m.astype(np.float64)
    l = r2.astype(_BF16)
    return h, m, l


def _build_pred_lhsT(pred, pp):
    """pred: [N, 2] fp32 -> lhsT [K, N] bf16 (stationary operand columns)."""
    px = pred[:, 0].astype(np.float64)
    py = pred[:, 1].astype(np.float64)
    pxh, pxm, pxl = _split3(px)
    pyh, pym, pyl = _split3(py)
    sp = px * px + py * py
    sph, spm, spl = _split3(sp)
    one = np.ones(N, dtype=_BF16)

    def n2(a):  # -2*a, exact in bf16
        return (-2.0 * a.astype(np.float64)).astype(_BF16)

    rows = [
        n2(pxh), n2(pxh), n2(pxm), n2(pxh), n2(pxl), n2(pxm),
        n2(pyh), n2(pyh), n2(pym), n2(pyh), n2(pyl), n2(pym),
        sph, spm, spl,
        one, one, one,
    ]
    return np.stack(rows, axis=0)


def _build_targ_rhs(targ, tp):
    """targ: [M, 2] fp32 -> rhs [K, M] bf16 (moving operand columns)."""
    tx = targ[:, 0].astype(np.float64)
    ty = targ[:, 1].astype(np.float64)
    txh, txm, txl = _split3(tx)
    tyh, tym, tyl = _split3(ty)
    q = tx * tx + ty * ty
    qh, qm, ql = _split3(q)
    # invalid targets (m >= tp): bake +BIG into the hi row, zero mid/lo
    invalid = np.arange(M) >= tp
    qh = qh.copy()
    qm = qm.copy()
    ql = ql.copy()
    qh[invalid] = _BF16(BIG)
    qm[invalid] = _BF16(0.0)
    ql[invalid] = _BF16(0.0)
    one = np.ones(M, dtype=_BF16)
    rows = [
        txh, txm, txh, txl, txh, txm,
        tyh, tym, tyh, tyl, tyh, tym,
        one, one, one,
        qh, qm, ql,
    ]
    return np.stack(rows, axis=0)


def _register_custom_op():
    """Register TT_MIN_MIN_ANT: out = min(in0, in1); accum_out = min-reduce.

    Lets the DVE drain one PSUM element and one SBUF element per lane-cycle
    with the full min-reduction fused in, halving the reduction bottleneck
    vs tensor_reduce (which ingests only 1 element/cycle).
    """
    import concourse.dve_ops as dvo
    from concourse.dve_spec import C0, Spec, Src0, Src1, lower, minn
    from concourse.dve_uop import DveOpSpec

    name = "TT_MIN_MIN_ANT"
    for existing in dvo.OPS:
        if existing.name == name:
            return existing
    spec = Spec(body=minn(Src0, Src1), accum=minn, accum_init=C0)
    row = max(dvo._SUB_OPCODE_FOR_NAME.values()) + 1
    assert row < 0x20
    dvo._SUB_OPCODE_FOR_NAME[name] = row
    sha = {}
    for ver in ("v3", "v4"):
        uops = lower(spec, ver=ver)
        sha[ver] = DveOpSpec(name=name, opcode=row, uops=uops, rd1_en=True).sha(
            ver
        )
    op = dvo.DveOp(name, spec, subdim=False, uops_sha=sha)
    dvo.OPS.append(op)
    dvo.CUSTOM_DVE_SPECS[name] = spec
    return op


def _build_bass():
    import concourse.bass as bass
    import concourse.mybir as mybir

    f32 = mybir.dt.float32
    bf16 = mybir.dt.bfloat16
    X = mybir.AxisListType.X
    ttr_op = _register_custom_op()

    nc = bass.Bass()
    predT = nc.declare_dram_parameter("predT", [K, N], bf16, isOutput=False)
    targT = nc.declare_dram_parameter("targT", [K, M], bf16, isOutput=False)
    pmask = nc.declare_dram_parameter("pmask", [128, NT + 1], f32, isOutput=False)
    out = nc.declare_dram_parameter("out", [128, 1], f32, isOutput=True)

    with (
        nc.sbuf_tensor([K, N], bf16) as predS,
        nc.sbuf_tensor([K, M], bf16) as targS,
        nc.sbuf_tensor([128, NT + 1], f32) as maskS,
        nc.sbuf_tensor([128, NT], f32) as minbuf,
        nc.sbuf_tensor([128, 2048], f32) as S_a,
        nc.sbuf_tensor([128, 2048], f32) as S_b,
        nc.sbuf_tensor([128, 1], f32) as dummy,
        nc.sbuf_tensor([128, NT], f32) as masked,
        nc.sbuf_tensor([128, 1], f32) as colsum,
        nc.psum_tensor([128, 4096], f32) as PS,
        nc.semaphore("dma_sem") as dma_sem,
        nc.semaphore("pe_sem") as pe_sem,
        nc.semaphore("act_sem") as act_sem,
        nc.semaphore("dve_sem") as dve_sem,
        nc.Block() as block,
    ):
        S = [S_a, S_b]

        @block.sync
        def _(sync):
            sync.dma_start(predS[:], predT[:]).then_inc(dma_sem, 16)
            sync.dma_start(targS[:], targT[:]).then_inc(dma_sem, 16)
            sync.dma_start(maskS[:], pmask[:]).then_inc(dma_sem, 16)
            # output DMA waits for the DVE epilogue (NT reduces + 2 tail ops)
            sync.wait_ge(dve_sem, NT + 2)
            sync.dma_start(out[:], colsum[:]).then_inc(dma_sem, 16)

        @block.tensor
        def _(pe):
            # All three input DMAs' 16 sub-increments interleave, so only
            # the full 48 guarantees pred+targ are completely resident.
            pe.wait_ge(dma_sem, 48)
            for nt in range(NT):
                lhs = predS[:, nt * 128 : (nt + 1) * 128]
                # high half (m 2048..4095) first: ScalarE stages it to SBUF
                if nt > 0:
                    pe.wait_ge(act_sem, nt)  # copy of tile nt-1 freed R23
                mm = None
                for c in range(4):
                    m0 = 2048 + c * 512
                    mm = pe.matmul(
                        PS[:, m0 : m0 + 512],
                        lhsT=lhs,
                        rhs=targS[:, m0 : m0 + 512],
                        start=True,
                        stop=True,
                    )
                mm.then_inc(pe_sem, 1)  # -> 2*nt + 1
                if nt > 0:
                    pe.wait_ge(dve_sem, nt)  # reduce of tile nt-1 freed R01
                for c in range(4):
                    m0 = c * 512
                    mm = pe.matmul(
                        PS[:, m0 : m0 + 512],
                        lhsT=lhs,
                        rhs=targS[:, m0 : m0 + 512],
                        start=True,
                        stop=True,
                    )
                mm.then_inc(pe_sem, 1)  # -> 2*nt + 2

        @block.scalar
        def _(act):
            for nt in range(NT):
                if nt >= 2:
                    # staging buffer S[nt%2] last read by reduce of tile nt-2
                    act.wait_ge(dve_sem, nt - 1)
                act.wait_ge(pe_sem, 2 * nt + 1)
                act.copy(S[nt % 2][:], PS[:, 2048:4096]).then_inc(act_sem, 1)

        @block.vector
        def _(dve):
            for nt in range(NT):
                dve.wait_ge(act_sem, nt + 1)
                dve.wait_ge(pe_sem, 2 * nt + 2)
                dve._custom_dve(
                    ttr_op,
                    out=dummy[:].broadcast_to((128, 2048)),
                    in0=PS[:, 0:2048],
                    in1=S[nt % 2][:],
                    s0=BIG,
                    accum_out=minbuf[:, nt : nt + 1],
                ).then_inc(dve_sem, 1)
            # epilogue: mask -> row-sum (host sums the 128 rows)
            dve.wait_ge(dma_sem, 48)  # mask resident
            dve.tensor_mul(masked[:], minbuf[:], maskS[:, 0:NT]).then_inc(
                dve_sem, 1
            )
            dve.tensor_reduce(
                colsum[:], masked[:], axis=X, op=mybir.AluOpType.add
            ).then_inc(dve_sem, 1)

    # Populate .instr bytes for InstISA subclasses (custom DVE op); walrus
    # rejects empty payloads with "ISA wrong length".
    mybir.codegen_inst_isa_subclasses(nc)
    return nc


def _get_compiled():
    global _compiled
    if _compiled is None:
        _compiled = _build_bass()
    return _compiled


def kernel(predicted_coords, predicted_points, target_coords, target_points):
    from concourse.bass_utils import run_bass_kernel_spmd

    pred = np.asarray(predicted_coords)
    pp = np.asarray(predicted_points)
    targ = np.asarray(target_coords)
    tp = np.asarray(target_points)

    nc = _get_compiled()
    core_ids = list(range(B))

    in_maps = []
    for b in range(B):
        lhsT = _build_pred_lhsT(pred[b], int(pp[b]))
        rhs = _build_targ_rhs(targ[b], int(tp[b]))
        pm = np.zeros((128, NT + 1), dtype=np.float32)
        n_idx = np.arange(N).reshape(NT, 128).T  # [128, NT]: n = nt*128 + p
        pm[:, :NT] = (n_idx < int(pp[b])).astype(np.float32)
        pm[:, NT] = 1.0  # ones column (rhs of the partition-sum matmul)
        in_maps.append({"predT": lhsT, "targT": rhs, "pmask": pm})

    results = run_bass_kernel_spmd(nc, in_maps, core_ids).results

    sums = np.array(
        [results[b]["out"].astype(np.float64).sum() for b in range(B)]
    )
    pp64 = pp.astype(np.float64)
    tp64 = tp.astype(np.float64)
    coord_b = sums / (pp64 * D)
    coord = coord_b.mean()
    points = (((pp64 - tp64) / N) ** 2).mean()
    total = coord + 0.1 * points
    return (
        np.float32(total),
        np.float32(coord),
        np.float32(points),
    )


# revision 12
# speedup vs baseline: 1.2759x; 1.2759x over previous
"""Trainium2 Bass kernel for nn_PointCloud2LaserScanLoss.

Problem (per batch element b of B=8):
    d2[n,m] = ||pred[n] - targ[m]||^2          (N=M=4096, D=2)
    minval[n] = min over valid m (m < tp_b) of d2[n,m]
    coord_b   = sum over valid n (n < pp_b) of minval[n] / (pp_b * D)
    coord     = mean_b coord_b
    points    = mean_b ((pp_b - tp_b)/N)^2
    total     = coord + 0.1 * points
(The reference gathers matched targets and recomputes the squared distance,
which is numerically the min of the masked distance row — so no argmin/gather
is needed, just a masked min-reduction.)

Sharding: data-parallel over batch; core b handles batch element b.

Device algorithm per core:
  - d2 is computed on the TensorEngine as an 18-row augmented matmul in bf16
    using an exact 3-way bf16 split of every coordinate (hi/mid/lo), which
    reproduces fp32-grade precision at full bf16 PE speed (fp32 matmul is 4x
    slower).  The target-validity mask is baked into the |t|^2 rows (1e30 for
    invalid targets).
  - The [128, 4096] distance rows per n-tile land in PSUM as four [128,1024]
    quarters.  ScalarE copies two quarters to SBUF; VectorE then runs
    tensor_tensor_reduce (elementwise min of a PSUM quarter and an SBUF
    quarter, fused with a min-reduction) so the DVE drains 2 elements per
    lane-cycle instead of 1.
  - Epilogue on device: pair-min, predicted-validity mask multiply,
    free-axis sum, then a [128,1]x[128,1] matmul to sum over partitions.
Host combines the 8 per-core masked sums into the three scalar losses.
"""

import sys

import numpy as np

if "/opt/trn_rl_repo" not in sys.path:
    sys.path.insert(0, "/opt/trn_rl_repo")

import ml_dtypes

B, N, M, D = 8, 4096, 4096, 2
NT = N // 128  # 32 n-tiles
K = 18  # augmented contraction rows
BIG = 1e30

_BF16 = ml_dtypes.bfloat16

_compiled = None  # cached (nc, core_ids)


def _split3(v64):
    """Exact-ish 3-way bf16 split: v ~= h + m + l with residual ~2^-27 |v|."""
    h = v64.astype(_BF16)
    r = v64 - h.astype(np.float64)
    m = r.astype(_BF16)
    r2 = r - m.astype(np.float64)
    l = r2.astype(_BF16)
    return h, m, l


def _build_pred_lhsT(pred, pp):
    """pred: [N, 2] fp32 -> lhsT [128, N] bf16, K=18 rows replicated into the
    four 32-row PE groups (partitions 32g..32g+17) for row-packed matmuls."""
    px = pred[:, 0].astype(np.float64)
    py = pred[:, 1].astype(np.float64)
    pxh, pxm, pxl = _split3(px)
    pyh, pym, pyl = _split3(py)
    sp = px * px + py * py
    sph, spm, spl = _split3(sp)
    one = np.ones(N, dtype=_BF16)

    def n2(a):  # -2*a, exact in bf16
        return (-2.0 * a.astype(np.float64)).astype(_BF16)

    rows = [
        n2(pxh), n2(pxh), n2(pxm), n2(pxh), n2(pxl), n2(pxm),
        n2(pyh), n2(pyh), n2(pym), n2(pyh), n2(pyl), n2(pym),
        sph, spm, spl,
        one, one, one,
    ]
    block = np.stack(rows, axis=0)
    full = np.zeros((128, N), dtype=_BF16)
    for g in range(4):
        full[32 * g : 32 * g + K] = block
    return full


def _build_targ_rhs(targ, tp):
    """targ: [M, 2] fp32 -> rhs [K, M] bf16 (moving operand columns)."""
    tx = targ[:, 0].astype(np.float64)
    ty = targ[:, 1].astype(np.float64)
    txh, txm, txl = _split3(tx)
    tyh, tym, tyl = _split3(ty)
    q = tx * tx + ty * ty
    qh, qm, ql = _split3(q)
    # invalid targets (m >= tp): bake +BIG into the hi row, zero mid/lo
    invalid = np.arange(M) >= tp
    qh = qh.copy()
    qm = qm.copy()
    ql = ql.copy()
    qh[invalid] = _BF16(BIG)
    qm[invalid] = _BF16(0.0)
    ql[invalid] = _BF16(0.0)
    one = np.ones(M, dtype=_BF16)
    rows = [
        txh, txm, txh, txl, txh, txm,
        tyh, tym, tyh, tyl, tyh, tym,
        one, one, one,
        qh, qm, ql,
    ]
    block = np.stack(rows, axis=0)
    full = np.zeros((128, M), dtype=_BF16)
    for g in range(4):
        full[32 * g : 32 * g + K] = block
    return full


def _register_custom_op():
    """Register TT_MIN_MIN_ANT: out = min(in0, in1); accum_out = min-reduce.

    Lets the DVE drain one PSUM element and one SBUF element per lane-cycle
    with the full min-reduction fused in, halving the reduction bottleneck
    vs tensor_reduce (which ingests only 1 element/cycle).
    """
    import concourse.dve_ops as dvo
    from concourse.dve_spec import C0, Spec, Src0, Src1, lower, minn
    from concourse.dve_uop import DveOpSpec

    name = "TT_MIN_MIN_ANT"
    for existing in dvo.OPS:
        if existing.name == name:
            return existing
    spec = Spec(body=minn(Src0, Src1), accum=minn, accum_init=C0)
    row = max(dvo._SUB_OPCODE_FOR_NAME.values()) + 1
    assert row < 0x20
    dvo._SUB_OPCODE_FOR_NAME[name] = row
    sha = {}
    for ver in ("v3", "v4"):
        uops = lower(spec, ver=ver)
        sha[ver] = DveOpSpec(name=name, opcode=row, uops=uops, rd1_en=True).sha(
            ver
        )
    op = dvo.DveOp(name, spec, subdim=False, uops_sha=sha)
    dvo.OPS.append(op)
    dvo.CUSTOM_DVE_SPECS[name] = spec
    return op


def _build_bass():
    import concourse.bass as bass
    import concourse.mybir as mybir

    f32 = mybir.dt.float32
    bf16 = mybir.dt.bfloat16
    X = mybir.AxisListType.X
    ttr_op = _register_custom_op()

    nc = bass.Bass()
    predT = nc.declare_dram_parameter("predT", [128, N], bf16, isOutput=False)
    targT = nc.declare_dram_parameter("targT", [128, M], bf16, isOutput=False)
    pmask = nc.declare_dram_parameter("pmask", [128, NT + 1], f32, isOutput=False)
    out = nc.declare_dram_parameter("out", [128, 1], f32, isOutput=True)

    with (
        nc.sbuf_tensor([128, N], bf16) as predS,
        nc.sbuf_tensor([128, M], bf16) as targS,
        nc.sbuf_tensor([128, NT + 1], f32) as maskS,
        nc.sbuf_tensor([128, NT], f32) as minbuf,
        nc.sbuf_tensor([128, 2048], f32) as S_a,
        nc.sbuf_tensor([128, 2048], f32) as S_b,
        nc.sbuf_tensor([128, 1], f32) as dummy,
        nc.sbuf_tensor([128, NT], f32) as masked,
        nc.sbuf_tensor([128, 1], f32) as colsum,
        nc.psum_tensor([128, 4096], f32) as PS,
        nc.semaphore("dma_sem") as dma_sem,
        nc.semaphore("pe_sem") as pe_sem,
        nc.semaphore("act_sem") as act_sem,
        nc.semaphore("dve_sem") as dve_sem,
        nc.Block() as block,
    ):
        S = [S_a, S_b]

        @block.sync
        def _(sync):
            sync.dma_start(predS[:], predT[:]).then_inc(dma_sem, 16)
            sync.dma_start(targS[:], targT[:]).then_inc(dma_sem, 16)
            sync.dma_start(maskS[:], pmask[:]).then_inc(dma_sem, 16)
            # output DMA waits for the DVE epilogue (NT reduces + 2 tail ops)
            sync.wait_ge(dve_sem, NT + 2)
            sync.dma_start(out[:], colsum[:]).then_inc(dma_sem, 16)

        @block.tensor
        def _(pe):
            # All three input DMAs' 16 sub-increments interleave, so only
            # the full 48 guarantees pred+targ are completely resident.
            pe.wait_ge(dma_sem, 48)
            for nt in range(NT):
                # Row-packed quads: the four 512-wide chunks of each half run
                # concurrently in the four 32-row PE groups (K=18 per group).
                # high half (m 2048..4095) first: ScalarE stages it to SBUF
                if nt > 0:
                    pe.wait_ge(act_sem, nt)  # copy of tile nt-1 freed R23
                mm = None
                for g in range(4):
                    m0 = 2048 + g * 512
                    rg = slice(32 * g, 32 * g + K)
                    mm = pe.matmul(
                        PS[:, m0 : m0 + 512],
                        lhsT=predS[rg, nt * 128 : (nt + 1) * 128],
                        rhs=targS[rg, m0 : m0 + 512],
                        start=True,
                        stop=True,
                        tile_position=(32 * g, 0),
                    )
                mm.then_inc(pe_sem, 1)  # -> 2*nt + 1
                if nt > 0:
                    pe.wait_ge(dve_sem, nt)  # reduce of tile nt-1 freed R01
                for g in range(4):
                    m0 = g * 512
                    rg = slice(32 * g, 32 * g + K)
                    mm = pe.matmul(
                        PS[:, m0 : m0 + 512],
                        lhsT=predS[rg, nt * 128 : (nt + 1) * 128],
                        rhs=targS[rg, m0 : m0 + 512],
                        start=True,
                        stop=True,
                        tile_position=(32 * g, 0),
                    )
                mm.then_inc(pe_sem, 1)  # -> 2*nt + 2

        @block.scalar
        def _(act):
            for nt in range(NT):
                if nt >= 2:
                    # staging buffer S[nt%2] last read by reduce of tile nt-2
                    act.wait_ge(dve_sem, nt - 1)
                act.wait_ge(pe_sem, 2 * nt + 1)
                act.copy(S[nt % 2][:], PS[:, 2048:4096]).then_inc(act_sem, 1)

        @block.vector
        def _(dve):
            for nt in range(NT):
                dve.wait_ge(act_sem, nt + 1)
                dve.wait_ge(pe_sem, 2 * nt + 2)
                dve._custom_dve(
                    ttr_op,
                    out=dummy[:].broadcast_to((128, 2048)),
                    in0=PS[:, 0:2048],
                    in1=S[nt % 2][:],
                    s0=BIG,
                    accum_out=minbuf[:, nt : nt + 1],
                ).then_inc(dve_sem, 1)
            # epilogue: mask -> row-sum (host sums the 128 rows)
            dve.wait_ge(dma_sem, 48)  # mask resident
            dve.tensor_mul(masked[:], minbuf[:], maskS[:, 0:NT]).then_inc(
                dve_sem, 1
            )
            dve.tensor_reduce(
                colsum[:], masked[:], axis=X, op=mybir.AluOpType.add
            ).then_inc(dve_sem, 1)

    # Populate .instr bytes for InstISA subclasses (custom DVE op); walrus
    # rejects empty payloads with "ISA wrong length".
    mybir.codegen_inst_isa_subclasses(nc)
    return nc


def _get_compiled():
    global _compiled
    if _compiled is None:
        _compiled = _build_bass()
    return _compiled


def kernel(predicted_coords, predicted_points, target_coords, target_points):
    from concourse.bass_utils import run_bass_kernel_spmd

    pred = np.asarray(predicted_coords)
    pp = np.asarray(predicted_points)
    targ = np.asarray(target_coords)
    tp = np.asarray(target_points)

    nc = _get_compiled()
    core_ids = list(range(B))

    in_maps = []
    for b in range(B):
        lhsT = _build_pred_lhsT(pred[b], int(pp[b]))
        rhs = _build_targ_rhs(targ[b], int(tp[b]))
        pm = np.zeros((128, NT + 1), dtype=np.float32)
        n_idx = np.arange(N).reshape(NT, 128).T  # [128, NT]: n = nt*128 + p
        pm[:, :NT] = (n_idx < int(pp[b])).astype(np.float32)
        pm[:, NT] = 1.0  # ones column (rhs of the partition-sum matmul)
        in_maps.append({"predT": lhsT, "targT": rhs, "pmask": pm})

    results = run_bass_kernel_spmd(nc, in_maps, core_ids).results

    sums = np.array(
        [results[b]["out"].astype(np.float64).sum() for b in range(B)]
    )
    pp64 = pp.astype(np.float64)
    tp64 = tp.astype(np.float64)
    coord_b = sums / (pp64 * D)
    coord = coord_b.mean()
    points = (((pp64 - tp64) / N) ** 2).mean()
    total = coord + 0.1 * points
    return (
        np.float32(total),
        np.float32(coord),
        np.float32(points),
    )


# revision 13
# speedup vs baseline: 1.6812x; 1.3176x over previous
"""Trainium2 Bass kernel for nn_PointCloud2LaserScanLoss.

Problem (per batch element b of B=8):
    d2[n,m] = ||pred[n] - targ[m]||^2          (N=M=4096, D=2)
    minval[n] = min over valid m (m < tp_b) of d2[n,m]
    coord_b   = sum over valid n (n < pp_b) of minval[n] / (pp_b * D)
    coord     = mean_b coord_b
    points    = mean_b ((pp_b - tp_b)/N)^2
    total     = coord + 0.1 * points
(The reference gathers matched targets and recomputes the squared distance,
which is numerically the min of the masked distance row — so no argmin/gather
is needed, just a masked min-reduction.)

Sharding: data-parallel over batch; core b handles batch element b.

Device algorithm per core:
  - d2 is computed on the TensorEngine as an 18-row augmented matmul in bf16
    using an exact 3-way bf16 split of every coordinate (hi/mid/lo), which
    reproduces fp32-grade precision at full bf16 PE speed (fp32 matmul is 4x
    slower).  The target-validity mask is baked into the |t|^2 rows (1e30 for
    invalid targets).
  - The [128, 4096] distance rows per n-tile land in PSUM as four [128,1024]
    quarters.  ScalarE copies two quarters to SBUF; VectorE then runs
    tensor_tensor_reduce (elementwise min of a PSUM quarter and an SBUF
    quarter, fused with a min-reduction) so the DVE drains 2 elements per
    lane-cycle instead of 1.
  - Epilogue on device: pair-min, predicted-validity mask multiply,
    free-axis sum, then a [128,1]x[128,1] matmul to sum over partitions.
Host combines the 8 per-core masked sums into the three scalar losses.
"""

import sys

import numpy as np

if "/opt/trn_rl_repo" not in sys.path:
    sys.path.insert(0, "/opt/trn_rl_repo")

import ml_dtypes

B, N, M, D = 8, 4096, 4096, 2
NT = N // 128  # 32 n-tiles
K = 18  # augmented contraction rows
BIG = 1e30

_BF16 = ml_dtypes.bfloat16

_compiled = None  # cached (nc, core_ids)


def _split3(v64):
    """Exact-ish 3-way bf16 split: v ~= h + m + l with residual ~2^-27 |v|."""
    h = v64.astype(_BF16)
    r = v64 - h.astype(np.float64)
    m = r.astype(_BF16)
    r2 = r - m.astype(np.float64)
    l = r2.astype(_BF16)
    return h, m, l


def _build_pred_lhsT(pred, pp):
    """pred: [N, 2] fp32 -> lhsT [128, N] bf16, K=18 rows replicated into the
    four 32-row PE groups (partitions 32g..32g+17) for row-packed matmuls."""
    px = pred[:, 0].astype(np.float64)
    py = pred[:, 1].astype(np.float64)
    pxh, pxm, pxl = _split3(px)
    pyh, pym, pyl = _split3(py)
    sp = px * px + py * py
    sph, spm, spl = _split3(sp)
    one = np.ones(N, dtype=_BF16)

    def n2(a):  # -2*a, exact in bf16
        return (-2.0 * a.astype(np.float64)).astype(_BF16)

    rows = [
        n2(pxh), n2(pxh), n2(pxm), n2(pxh), n2(pxl), n2(pxm),
        n2(pyh), n2(pyh), n2(pym), n2(pyh), n2(pyl), n2(pym),
        sph, spm, spl,
        one, one, one,
    ]
    block = np.stack(rows, axis=0)
    full = np.zeros((128, N), dtype=_BF16)
    for g in range(4):
        full[32 * g : 32 * g + K] = block
    return full


def _build_targ_rhs(targ, tp):
    """targ: [M, 2] fp32 -> rhs [K, M] bf16 (moving operand columns)."""
    tx = targ[:, 0].astype(np.float64)
    ty = targ[:, 1].astype(np.float64)
    txh, txm, txl = _split3(tx)
    tyh, tym, tyl = _split3(ty)
    q = tx * tx + ty * ty
    qh, qm, ql = _split3(q)
    # invalid targets (m >= tp): bake +BIG into the hi row, zero mid/lo
    invalid = np.arange(M) >= tp
    qh = qh.copy()
    qm = qm.copy()
    ql = ql.copy()
    qh[invalid] = _BF16(BIG)
    qm[invalid] = _BF16(0.0)
    ql[invalid] = _BF16(0.0)
    one = np.ones(M, dtype=_BF16)
    rows = [
        txh, txm, txh, txl, txh, txm,
        tyh, tym, tyh, tyl, tyh, tym,
        one, one, one,
        qh, qm, ql,
    ]
    block = np.stack(rows, axis=0)
    full = np.zeros((128, M), dtype=_BF16)
    for g in range(4):
        full[32 * g : 32 * g + K] = block
    return full


def _register_custom_op():
    """Register TT_MIN_MIN_ANT: out = min(in0, in1); accum_out = min-reduce.

    Lets the DVE drain one PSUM element and one SBUF element per lane-cycle
    with the full min-reduction fused in, halving the reduction bottleneck
    vs tensor_reduce (which ingests only 1 element/cycle).
    """
    import concourse.dve_ops as dvo
    from concourse.dve_spec import C0, Spec, Src0, Src1, lower, minn
    from concourse.dve_uop import DveOpSpec

    name = "TT_MIN_MIN_ANT"
    for existing in dvo.OPS:
        if existing.name == name:
            return existing
    spec = Spec(body=minn(Src0, Src1), accum=minn, accum_init=C0)
    row = max(dvo._SUB_OPCODE_FOR_NAME.values()) + 1
    assert row < 0x20
    dvo._SUB_OPCODE_FOR_NAME[name] = row
    sha = {}
    for ver in ("v3", "v4"):
        uops = lower(spec, ver=ver)
        sha[ver] = DveOpSpec(name=name, opcode=row, uops=uops, rd1_en=True).sha(
            ver
        )
    op = dvo.DveOp(name, spec, subdim=False, uops_sha=sha)
    dvo.OPS.append(op)
    dvo.CUSTOM_DVE_SPECS[name] = spec
    return op


def _build_bass():
    import concourse.bass as bass
    import concourse.mybir as mybir

    f32 = mybir.dt.float32
    bf16 = mybir.dt.bfloat16
    X = mybir.AxisListType.X
    ttr_op = _register_custom_op()

    nc = bass.Bass()
    predT = nc.declare_dram_parameter("predT", [128, N], bf16, isOutput=False)
    targT = nc.declare_dram_parameter("targT", [128, M], bf16, isOutput=False)
    pmask = nc.declare_dram_parameter("pmask", [128, NT + 1], f32, isOutput=False)
    out = nc.declare_dram_parameter("out", [128, 1], f32, isOutput=True)

    # PSUM regions (1024 fp32 = 2 banks each):
    #   R0 = [0:1024), R1 = [1024:2048)    -> read directly by the DVE TTRs
    #   R2 = [2048:3072), R3 = [3072:4096) -> staged to SBUF by ScalarE
    # Each region is produced by a pack of 2 row-group-concurrent matmuls
    # and consumed by exactly one reader, so every refill happens off the
    # DVE critical path.
    with (
        nc.sbuf_tensor([128, N], bf16) as predS,
        nc.sbuf_tensor([128, M], bf16) as targS,
        nc.sbuf_tensor([128, NT + 1], f32) as maskS,
        nc.sbuf_tensor([128, 2 * NT], f32) as minbuf,
        nc.sbuf_tensor([128, 1024], f32) as S_a0,
        nc.sbuf_tensor([128, 1024], f32) as S_a1,
        nc.sbuf_tensor([128, 1024], f32) as S_b0,
        nc.sbuf_tensor([128, 1024], f32) as S_b1,
        nc.sbuf_tensor([128, 1], f32) as dummy,
        nc.sbuf_tensor([128, NT], f32) as red,
        nc.sbuf_tensor([128, NT], f32) as masked,
        nc.sbuf_tensor([128, 1], f32) as colsum,
        nc.psum_tensor([128, 4096], f32) as PS,
        nc.semaphore("dma_sem") as dma_sem,
        nc.semaphore("pe_sem") as pe_sem,
        nc.semaphore("act_sem") as act_sem,
        nc.semaphore("dve_sem") as dve_sem,
        nc.Block() as block,
    ):
        S_a = [S_a0, S_a1]
        S_b = [S_b0, S_b1]

        def pack(pe, nt, pair, m_base):
            """Two row-group-concurrent matmuls covering [m_base, m_base+1024)."""
            mm = None
            for j in range(2):
                g = 2 * pair + j
                m0 = m_base + j * 512
                rg = slice(32 * g, 32 * g + K)
                mm = pe.matmul(
                    PS[:, m0 : m0 + 512],
                    lhsT=predS[rg, nt * 128 : (nt + 1) * 128],
                    rhs=targS[rg, m0 : m0 + 512],
                    start=True,
                    stop=True,
                    tile_position=(32 * g, 0),
                )
            return mm

        @block.sync
        def _(sync):
            sync.dma_start(predS[:], predT[:]).then_inc(dma_sem, 16)
            sync.dma_start(targS[:], targT[:]).then_inc(dma_sem, 16)
            sync.dma_start(maskS[:], pmask[:]).then_inc(dma_sem, 16)
            sync.wait_ge(dve_sem, 2 * NT + 3)
            sync.dma_start(out[:], colsum[:]).then_inc(dma_sem, 16)

        @block.tensor
        def _(pe):
            # All three input DMAs' 16 sub-increments interleave, so only
            # the full 48 guarantees pred+targ are completely resident.
            pe.wait_ge(dma_sem, 48)
            for nt in range(NT):
                if nt > 0:
                    pe.wait_ge(act_sem, 2 * (nt - 1) + 1)  # copy_a freed R2
                pack(pe, nt, 0, 2048).then_inc(pe_sem, 1)  # -> 4nt+1
                if nt > 0:
                    pe.wait_ge(act_sem, 2 * (nt - 1) + 2)  # copy_b freed R3
                pack(pe, nt, 1, 3072).then_inc(pe_sem, 1)  # -> 4nt+2
                if nt > 0:
                    pe.wait_ge(dve_sem, 2 * (nt - 1) + 1)  # TTR_a freed R0
                pack(pe, nt, 0, 0).then_inc(pe_sem, 1)  # -> 4nt+3
                if nt > 0:
                    pe.wait_ge(dve_sem, 2 * (nt - 1) + 2)  # TTR_b freed R1
                pack(pe, nt, 1, 1024).then_inc(pe_sem, 1)  # -> 4nt+4

        @block.scalar
        def _(act):
            for nt in range(NT):
                if nt >= 2:
                    act.wait_ge(dve_sem, 2 * (nt - 2) + 1)  # S_a[nt%2] free
                act.wait_ge(pe_sem, 4 * nt + 1)
                act.copy(S_a[nt % 2][:], PS[:, 2048:3072]).then_inc(act_sem, 1)
                if nt >= 2:
                    act.wait_ge(dve_sem, 2 * (nt - 2) + 2)  # S_b[nt%2] free
                act.wait_ge(pe_sem, 4 * nt + 2)
                act.copy(S_b[nt % 2][:], PS[:, 3072:4096]).then_inc(act_sem, 1)

        @block.vector
        def _(dve):
            for nt in range(NT):
                dve.wait_ge(act_sem, 2 * nt + 1)
                dve.wait_ge(pe_sem, 4 * nt + 3)
                dve._custom_dve(
                    ttr_op,
                    out=dummy[:].broadcast_to((128, 1024)),
                    in0=PS[:, 0:1024],
                    in1=S_a[nt % 2][:],
                    s0=BIG,
                    accum_out=minbuf[:, 2 * nt : 2 * nt + 1],
                ).then_inc(dve_sem, 1)
                dve.wait_ge(act_sem, 2 * nt + 2)
                dve.wait_ge(pe_sem, 4 * nt + 4)
                dve._custom_dve(
                    ttr_op,
                    out=dummy[:].broadcast_to((128, 1024)),
                    in0=PS[:, 1024:2048],
                    in1=S_b[nt % 2][:],
                    s0=BIG,
                    accum_out=minbuf[:, 2 * nt + 1 : 2 * nt + 2],
                ).then_inc(dve_sem, 1)
            # epilogue: pair-min -> mask -> row-sum (host sums the 128 rows)
            dve.tensor_reduce(
                red[:],
                minbuf[:].rearrange("p (nt two) -> p nt two", two=2),
                axis=X,
                op=mybir.AluOpType.min,
            ).then_inc(dve_sem, 1)
            dve.wait_ge(dma_sem, 48)  # mask resident
            dve.tensor_mul(masked[:], red[:], maskS[:, 0:NT]).then_inc(dve_sem, 1)
            dve.tensor_reduce(
                colsum[:], masked[:], axis=X, op=mybir.AluOpType.add
            ).then_inc(dve_sem, 1)

    # Populate .instr bytes for InstISA subclasses (custom DVE op); walrus
    # rejects empty payloads with "ISA wrong length".
    mybir.codegen_inst_isa_subclasses(nc)
    return nc


def _get_compiled():
    global _compiled
    if _compiled is None:
        _compiled = _build_bass()
    return _compiled


def kernel(predicted_coords, predicted_points, target_coords, target_points):
    from concourse.bass_utils import run_bass_kernel_spmd

    pred = np.asarray(predicted_coords)
    pp = np.asarray(predicted_points)
    targ = np.asarray(target_coords)
    tp = np.asarray(target_points)

    nc = _get_compiled()
    core_ids = list(range(B))

    in_maps = []
    for b in range(B):
        lhsT = _build_pred_lhsT(pred[b], int(pp[b]))
        rhs = _build_targ_rhs(targ[b], int(tp[b]))
        pm = np.zeros((128, NT + 1), dtype=np.float32)
        n_idx = np.arange(N).reshape(NT, 128).T  # [128, NT]: n = nt*128 + p
        pm[:, :NT] = (n_idx < int(pp[b])).astype(np.float32)
        pm[:, NT] = 1.0  # ones column (rhs of the partition-sum matmul)
        in_maps.append({"predT": lhsT, "targT": rhs, "pmask": pm})

    results = run_bass_kernel_spmd(nc, in_maps, core_ids).results

    sums = np.array(
        [results[b]["out"].astype(np.float64).sum() for b in range(B)]
    )
    pp64 = pp.astype(np.float64)
    tp64 = tp.astype(np.float64)
    coord_b = sums / (pp64 * D)
    coord = coord_b.mean()
    points = (((pp64 - tp64) / N) ** 2).mean()
    total = coord + 0.1 * points
    return (
        np.float32(total),
        np.float32(coord),
        np.float32(points),
    )
